# revision 8
# baseline (speedup 1.0000x reference)
"""Trainium2 Bass kernel for per-sample dynamic conv (SE-attention mixed 3x3 kernels).

Computation (per sample b):
    att[b, :]  = sigmoid(gn(mish(gn(mean_hw(x[b]) @ w1.T)) @ w2.T))   # [K]
    agg_w[b]   = sum_k att[b,k] * weight[k]                           # [C,C,3,3]
    agg_b[b]   = att[b, :] @ bias                                     # [C]
    out[b]     = conv2d(x[b], agg_w[b], padding=1) + agg_b[b]

Strategy: pure data parallel over batch on 8 NeuronCores (4 samples each).
Conv is done as 9 shifted bf16 matmuls per 8-row output block accumulating in
PSUM; x is staged host-side zero-padded to 66-wide rows so the shifts never
need edge fixups. Weights are staged host-side pre-transposed to [ci, tap, co]
(matmul lhsT layout). SE path runs in f32 on-chip.
"""

import os
import numpy as np
import ml_dtypes

BS, C, HH, WW = 32, 128, 64, 64
K, HID = 4, 8
N_CORES = 8
BSL = BS // N_CORES          # samples per core
LROW = WW + 2                # 66: row pitch with left/right zero pad
LPAD = HH * LROW + 2         # 4226: padded per-channel length (leading+trailing zero)
NTAP = 9
RB = 8                       # output rows per PSUM block
NBLK = HH // RB              # 8 blocks
EPS = 1e-5

_cache = {}

# exec time (ns) of the last hardware run, when tracing was enabled
LAST_EXEC_NS = None


def _install_trace_hook():
    """Make run_bass_kernel_spmd(trace=True) work under axon by supplying the
    missing antenv.axon_hooks module (NTFF profiling via libaxon ctypes)."""
    import sys, types
    if "antenv.axon_hooks" in sys.modules:
        return True
    try:
        from trn_agent_boot.trn_boot import _ntff_profile_via_ctypes
        hook = _ntff_profile_via_ctypes("/opt/axon/libaxon_pjrt.so")
    except Exception:
        return False
    m = types.ModuleType("antenv.axon_hooks")
    m.get_axon_ntff_profile_hook = lambda: hook
    m.set_axon_ntff_profile_hook = lambda h: None
    sys.modules["antenv.axon_hooks"] = m
    return True


def _build_nc():
    import concourse.bass as bass
    import concourse.tile as tile
    from concourse import bacc, mybir

    f32 = mybir.dt.float32
    bf16 = mybir.dt.bfloat16
    Alu = mybir.AluOpType
    Act = mybir.ActivationFunctionType

    nc = bacc.Bacc("TRN2", target_bir_lowering=False, debug=False,
                   enable_asserts=True, num_devices=N_CORES)

    xp_d = nc.dram_tensor("xp", [BSL, C, LPAD], bf16, kind="ExternalInput").ap()
    wt_d = nc.dram_tensor("wt", [K, C, NTAP * C], bf16, kind="ExternalInput").ap()
    se1_d = nc.dram_tensor("se1", [C, HID], f32, kind="ExternalInput").ap()
    se2_d = nc.dram_tensor("se2", [HID, K], f32, kind="ExternalInput").ap()
    bias_d = nc.dram_tensor("biasT", [K, C], f32, kind="ExternalInput").ap()
    eye_d = nc.dram_tensor("eye4", [K, K], f32, kind="ExternalInput").ap()
    gn1s_d = nc.dram_tensor("gn1s", [1, HID], f32, kind="ExternalInput").ap()
    gn1b_d = nc.dram_tensor("gn1b", [1, HID], f32, kind="ExternalInput").ap()
    gn2s_d = nc.dram_tensor("gn2s", [1, K], f32, kind="ExternalInput").ap()
    gn2b_d = nc.dram_tensor("gn2b", [1, K], f32, kind="ExternalInput").ap()
    out_d = nc.dram_tensor("out", [BSL, C, HH * WW], bf16, kind="ExternalOutput").ap()

    with tile.TileContext(nc) as tc:
        with (
            tc.tile_pool(name="xin", bufs=BSL) as xpool,
            tc.tile_pool(name="wts", bufs=1) as wpool,
            tc.tile_pool(name="small", bufs=1) as spool,
            tc.tile_pool(name="agg", bufs=2) as apool,
            tc.tile_pool(name="ostage", bufs=4) as opool,
            tc.tile_pool(name="psmall", bufs=2, space="PSUM") as pspool,
            tc.tile_pool(name="pconv", bufs=6, space="PSUM") as cpool,
        ):
            # ---- load inputs ----
            x_tiles = []
            for b in range(BSL):
                xt = xpool.tile([C, LPAD], bf16, tag=f"x{b}")
                nc.sync.dma_start(xt[:], xp_d[b])
                x_tiles.append(xt)

            wt_all = wpool.tile([C, K * NTAP * C], bf16)
            for k in range(K):
                nc.sync.dma_start(wt_all[:, k * NTAP * C:(k + 1) * NTAP * C], wt_d[k])

            se1_sb = spool.tile([C, HID], f32, tag="se1")
            nc.sync.dma_start(se1_sb[:], se1_d[:])
            se2_sb = spool.tile([HID, K], f32, tag="se2")
            nc.sync.dma_start(se2_sb[:], se2_d[:])
            bias_sb = spool.tile([K, C], f32, tag="biasT")
            nc.sync.dma_start(bias_sb[:], bias_d[:])
            eye_sb = spool.tile([K, K], f32, tag="eye4")
            nc.sync.dma_start(eye_sb[:], eye_d[:])
            gn1s_sb = spool.tile([1, HID], f32, tag="gn1s")
            nc.sync.dma_start(gn1s_sb[:], gn1s_d[:])
            gn1b_sb = spool.tile([1, HID], f32, tag="gn1b")
            nc.sync.dma_start(gn1b_sb[:], gn1b_d[:])
            gn2s_sb = spool.tile([1, K], f32, tag="gn2s")
            nc.sync.dma_start(gn2s_sb[:], gn2s_d[:])
            gn2b_sb = spool.tile([1, K], f32, tag="gn2b")
            nc.sync.dma_start(gn2b_sb[:], gn2b_d[:])

            # ---- SE attention ----
            eps_sb = spool.tile([BSL, 1], f32, tag="eps")
            nc.vector.memset(eps_sb[:], EPS)

            # pooled_T[ci, b] = sum_hw x[b, ci] (zeros in padding don't affect sum;
            # the 1/4096 mean factor is pre-folded into se1 host-side)
            pooled = spool.tile([C, BSL], f32, tag="pooled")
            for b in range(BSL):
                nc.vector.tensor_reduce(
                    out=pooled[:, b:b + 1], in_=x_tiles[b][:],
                    axis=mybir.AxisListType.X, op=Alu.add)

            def groupnorm_rows(h_ps, n, scale_sb, bias_sb_, tag):
                """GroupNorm(1) over the free dim of a [BSL, n] PSUM tile.
                Returns normalized [BSL, n] f32 SBUF tile (pre-activation)."""
                msum = spool.tile([BSL, 1], f32, tag=f"{tag}_msum")
                nc.vector.tensor_reduce(out=msum[:], in_=h_ps[:],
                                        axis=mybir.AxisListType.X, op=Alu.add)
                mean = spool.tile([BSL, 1], f32, tag=f"{tag}_mean")
                nc.vector.tensor_scalar_mul(mean[:], msum[:], 1.0 / n)
                cent = spool.tile([BSL, n], f32, tag=f"{tag}_cent")
                nc.vector.tensor_scalar_sub(cent[:], h_ps[:], mean[:, 0:1])
                sq = spool.tile([BSL, n], f32, tag=f"{tag}_sq")
                vsum = spool.tile([BSL, 1], f32, tag=f"{tag}_vsum")
                nc.scalar.activation(sq[:], cent[:], Act.Square, accum_out=vsum[:])
                # 1/sqrt(var+eps) as exp(-0.5*ln(var+eps)): keeps the whole SE
                # path on one activation table (exp/ln/square) — Sqrt isn't in it
                lnv = spool.tile([BSL, 1], f32, tag=f"{tag}_lnv")
                nc.scalar.activation(lnv[:], vsum[:], Act.Ln,
                                     bias=eps_sb[:, 0:1], scale=1.0 / n)
                rstd = spool.tile([BSL, 1], f32, tag=f"{tag}_rstd")
                nc.scalar.activation(rstd[:], lnv[:], Act.Exp, scale=-0.5)
                norm = spool.tile([BSL, n], f32, tag=f"{tag}_norm")
                nc.vector.tensor_scalar_mul(norm[:], cent[:], rstd[:, 0:1])
                # elementwise affine along free dim (broadcast params to BSL partitions)
                s_bc = spool.tile([BSL, n], f32, tag=f"{tag}_sbc")
                nc.gpsimd.partition_broadcast(s_bc[:], scale_sb[:])
                b_bc = spool.tile([BSL, n], f32, tag=f"{tag}_bbc")
                nc.gpsimd.partition_broadcast(b_bc[:], bias_sb_[:])
                affn = spool.tile([BSL, n], f32, tag=f"{tag}_affn")
                nc.vector.tensor_mul(affn[:], norm[:], s_bc[:])
                affnb = spool.tile([BSL, n], f32, tag=f"{tag}_affnb")
                nc.vector.tensor_add(affnb[:], affn[:], b_bc[:])
                return affnb

            # h1 = pooled @ (se_w1/4096).T  -> [BSL, HID] (f32 matmul, tiny)
            h1_ps = pspool.tile([BSL, HID], f32, tag="seps")
            nc.tensor.matmul(h1_ps[:], lhsT=pooled[:], rhs=se1_sb[:])
            h1n = groupnorm_rows(h1_ps, HID, gn1s_sb, gn1b_sb, "gn1")

            # mish(v) = v * tanh(softplus(v)) = v * (1 - 2/((1+e^v)^2 + 1))
            # (composed from exp/square + DVE reciprocal; no tanh table needed)
            ev = spool.tile([BSL, HID], f32, tag="mish_ev")
            nc.scalar.activation(ev[:], h1n[:], Act.Exp)
            ep1 = spool.tile([BSL, HID], f32, tag="mish_ep1")
            nc.vector.tensor_scalar_add(ep1[:], ev[:], 1.0)
            q = spool.tile([BSL, HID], f32, tag="mish_q")
            nc.scalar.activation(q[:], ep1[:], Act.Square)
            qp1 = spool.tile([BSL, HID], f32, tag="mish_qp1")
            nc.vector.tensor_scalar_add(qp1[:], q[:], 1.0)
            rq = spool.tile([BSL, HID], f32, tag="mish_rq")
            nc.vector.reciprocal(rq[:], qp1[:])
            th = spool.tile([BSL, HID], f32, tag="mish_th")
            nc.vector.tensor_scalar(th[:], rq[:], -2.0, 1.0,
                                    op0=Alu.mult, op1=Alu.add)
            h1m = spool.tile([BSL, HID], f32, tag="h1m")
            nc.vector.tensor_mul(h1m[:], h1n[:], th[:])

            # h2 = h1m @ se_w2.T -> [BSL, K]; needs h1m.T as lhsT
            h1mT_ps = pspool.tile([HID, BSL], f32, tag="seps")
            nc.tensor.transpose(h1mT_ps[:], h1m[:], eye_sb[:])
            h1mT = spool.tile([HID, BSL], f32, tag="h1mT")
            nc.scalar.copy(h1mT[:], h1mT_ps[:])
            h2_ps = pspool.tile([BSL, K], f32, tag="seps")
            nc.tensor.matmul(h2_ps[:], lhsT=h1mT[:], rhs=se2_sb[:])
            h2n = groupnorm_rows(h2_ps, K, gn2s_sb, gn2b_sb, "gn2")
            # sigmoid(z) = 1 / (1 + e^-z)
            enz = spool.tile([BSL, K], f32, tag="att_enz")
            nc.scalar.activation(enz[:], h2n[:], Act.Exp, scale=-1.0)
            enz1 = spool.tile([BSL, K], f32, tag="att_enz1")
            nc.vector.tensor_scalar_add(enz1[:], enz[:], 1.0)
            att = spool.tile([BSL, K], f32, tag="att")
            nc.vector.reciprocal(att[:], enz1[:])

            # agg_b.T[co, b] = bias.T @ att.T  (lhsT = bias [k, co], rhs = att.T [k, b])
            attT_ps = pspool.tile([K, BSL], f32, tag="seps")
            nc.tensor.transpose(attT_ps[:], att[:], eye_sb[:])
            attT = spool.tile([K, BSL], f32, tag="attT")
            nc.scalar.copy(attT[:], attT_ps[:])
            aggb_ps = pspool.tile([C, BSL], f32, tag="seps")
            nc.tensor.matmul(aggb_ps[:], lhsT=bias_sb[:], rhs=attT[:])
            aggb = spool.tile([C, BSL], f32, tag="aggb")
            nc.scalar.copy(aggb[:], aggb_ps[:])

            # broadcast att to all 128 partitions: att_bc[p, b*K+k] = att[b, k]
            att_flat = spool.tile([1, BSL * K], f32, tag="attflat")
            nc.sync.dma_start(att_flat[:], att[:])
            att_bc = spool.tile([C, BSL * K], f32, tag="attbc")
            nc.gpsimd.partition_broadcast(att_bc[:], att_flat[:])

            # ---- per-sample aggregation + conv ----
            W1 = NTAP * C  # 1152 free elems per base kernel
            for b in range(BSL):
                # agg[ci, tap*C+co] = sum_k att[b,k] * wT[k][ci, tap*C+co]
                prev = None
                for k in range(K):
                    cur = apool.tile([C, W1], bf16, tag=f"agg{k}")
                    sc = att_bc[:, b * K + k:b * K + k + 1]
                    wk = wt_all[:, k * W1:(k + 1) * W1]
                    if prev is None:
                        nc.vector.tensor_scalar_mul(cur[:], wk, sc)
                    else:
                        nc.vector.scalar_tensor_tensor(
                            out=cur[:], in0=wk, scalar=sc, in1=prev[:],
                            op0=Alu.mult, op1=Alu.add)
                    prev = cur
                agg = prev

                for blk in range(NBLK):
                    h0 = blk * RB
                    ps = cpool.tile([C, RB * WW], f32, tag="convps")
                    ti = 0
                    taps = [(dh, dw) for dh in (0, -1, 1) for dw in (-1, 0, 1)]
                    for (dh, dw) in taps:
                        tt = 1 if h0 + dh < 0 else 0
                        bt = 1 if h0 + RB - 1 + dh > HH - 1 else 0
                        nr = RB - tt - bt
                        tap = (dh + 1) * 3 + (dw + 1)
                        start = 1 + (h0 + tt + dh) * LROW + dw
                        rhs = (x_tiles[b][:, start:start + nr * LROW]
                               .rearrange("p (r c) -> p r c", c=LROW)[:, :, 0:WW])
                        nc.tensor.matmul(
                            ps[:, tt * WW:(tt + nr) * WW],
                            lhsT=agg[:, tap * C:(tap + 1) * C],
                            rhs=rhs,
                            start=(ti == 0), stop=(ti == NTAP - 1))
                        ti += 1
                    osb = opool.tile([C, RB * WW], bf16)
                    nc.scalar.activation(osb[:], ps[:], Act.Identity,
                                         bias=aggb[:, b:b + 1], scale=1.0)
                    nc.sync.dma_start(out_d[b][:, h0 * WW:(h0 + RB) * WW], osb[:])

    nc.compile()
    return nc


def _stage_inputs(x, weight, bias, se_w1, gn1_scale, gn1_bias, se_w2,
                  gn2_scale, gn2_bias):
    """Host-side layout staging: shard, pad, transpose, cast. Returns in_maps."""
    bf16 = ml_dtypes.bfloat16

    # zero-padded x: per (b, ci) buffer of length LPAD; element (h, w) lives at
    # 1 + h*LROW + w, so w-1/w+64 shifts read zeros and row shifts stay in bounds.
    xp = np.zeros((BS, C, LPAD), dtype=bf16)
    xp_view = xp[:, :, 1:1 + HH * LROW].reshape(BS, C, HH, LROW)
    xp_view[:, :, :, :WW] = x.astype(bf16)

    # weight [k, o, i, h, w] -> lhsT layout [k, i, (h*3+w)*C + o]
    wt = np.ascontiguousarray(weight.transpose(0, 2, 3, 4, 1)).reshape(K, C, NTAP * C).astype(bf16)

    se1 = np.ascontiguousarray((se_w1 / float(HH * WW)).T).astype(np.float32)   # [C, HID]
    se2 = np.ascontiguousarray(se_w2.T).astype(np.float32)                      # [HID, K]
    biasT = np.ascontiguousarray(bias).astype(np.float32)                       # [K, C]
    eye4 = np.eye(K, dtype=np.float32)
    g1s = gn1_scale.reshape(1, HID).astype(np.float32)
    g1b = gn1_bias.reshape(1, HID).astype(np.float32)
    g2s = gn2_scale.reshape(1, K).astype(np.float32)
    g2b = gn2_bias.reshape(1, K).astype(np.float32)

    in_maps = []
    for i in range(N_CORES):
        in_maps.append({
            "xp": np.ascontiguousarray(xp[i * BSL:(i + 1) * BSL]),
            "wt": wt, "se1": se1, "se2": se2, "biasT": biasT, "eye4": eye4,
            "gn1s": g1s, "gn1b": g1b, "gn2s": g2s, "gn2b": g2b,
        })
    return in_maps


def kernel(x, weight, bias, se_w1, gn1_scale, gn1_bias, se_w2, gn2_scale,
           gn2_bias):
    global LAST_EXEC_NS
    x = np.asarray(x, dtype=np.float32)
    weight = np.asarray(weight, dtype=np.float32)
    bias = np.asarray(bias, dtype=np.float32)
    se_w1 = np.asarray(se_w1, dtype=np.float32)
    gn1_scale = np.asarray(gn1_scale, dtype=np.float32)
    gn1_bias = np.asarray(gn1_bias, dtype=np.float32)
    se_w2 = np.asarray(se_w2, dtype=np.float32)
    gn2_scale = np.asarray(gn2_scale, dtype=np.float32)
    gn2_bias = np.asarray(gn2_bias, dtype=np.float32)

    if "nc" not in _cache:
        _cache["nc"] = _build_nc()
    nc = _cache["nc"]

    in_maps = _stage_inputs(x, weight, bias, se_w1, gn1_scale, gn1_bias,
                            se_w2, gn2_scale, gn2_bias)

    trace = bool(int(os.environ.get("BASS_KERNEL_TRACE", "0")))
    if trace:
        trace = _install_trace_hook()

    from concourse.bass_utils import run_bass_kernel_spmd
    res = run_bass_kernel_spmd(nc, in_maps, core_ids=list(range(N_CORES)),
                               trace=trace)
    LAST_EXEC_NS = res.exec_time_ns

    out = np.empty((BS, C, HH, WW), dtype=np.float32)
    for i in range(N_CORES):
        out[i * BSL:(i + 1) * BSL] = (
            res.results[i]["out"].astype(np.float32).reshape(BSL, C, HH, WW))
    return out


# revision 24
# speedup vs baseline: 1.1739x; 1.1739x over previous
"""Trainium2 Bass kernel for per-sample dynamic conv (SE-attention mixed 3x3 kernels).

Computation (per sample b):
    att[b, :]  = sigmoid(gn(mish(gn(mean_hw(x[b]) @ w1.T)) @ w2.T))   # [K]
    agg_w[b]   = sum_k att[b,k] * weight[k]                           # [C,C,3,3]
    agg_b[b]   = att[b, :] @ bias                                     # [C]
    out[b]     = conv2d(x[b], agg_w[b], padding=1) + agg_b[b]

Strategy: pure data parallel over batch on 8 NeuronCores (4 samples each).
Conv is done as 9 shifted bf16 matmuls per 8-row output block accumulating in
PSUM; x is staged host-side zero-padded to 66-wide rows so the shifts never
need edge fixups. Weights are staged host-side pre-transposed to [ci, tap, co]
(matmul lhsT layout). SE path runs in f32 on-chip.
"""

import os
import numpy as np
import ml_dtypes

BS, C, HH, WW = 32, 128, 64, 64
K, HID = 4, 8
N_CORES = 8
BSL = BS // N_CORES          # samples per core
LROW = WW + 2                # 66: row pitch with left/right zero pad
LPAD = HH * LROW + 2         # 4226: padded per-channel length (leading+trailing zero)
NTAP = 9
RB = 8                       # output rows per PSUM block
NBLK = HH // RB              # 8 blocks
EPS = 1e-5

_cache = {}

# exec time (ns) of the last hardware run, when tracing was enabled
LAST_EXEC_NS = None


def _install_trace_hook():
    """Make run_bass_kernel_spmd(trace=True) work under axon by supplying the
    missing antenv.axon_hooks module (NTFF profiling via libaxon ctypes)."""
    import sys, types
    if "antenv.axon_hooks" in sys.modules:
        return True
    try:
        from trn_agent_boot.trn_boot import _ntff_profile_via_ctypes
        hook = _ntff_profile_via_ctypes("/opt/axon/libaxon_pjrt.so")
    except Exception:
        return False
    m = types.ModuleType("antenv.axon_hooks")
    m.get_axon_ntff_profile_hook = lambda: hook
    m.set_axon_ntff_profile_hook = lambda h: None
    sys.modules["antenv.axon_hooks"] = m
    return True


def _build_nc():
    import concourse.bass as bass
    import concourse.tile as tile
    from concourse import bacc, mybir

    f32 = mybir.dt.float32
    bf16 = mybir.dt.bfloat16
    Alu = mybir.AluOpType
    Act = mybir.ActivationFunctionType

    nc = bacc.Bacc("TRN2", target_bir_lowering=False, debug=False,
                   enable_asserts=True, num_devices=N_CORES)

    xp_d = nc.dram_tensor("xp", [BSL, C, LPAD], bf16, kind="ExternalInput").ap()
    wt_d = nc.dram_tensor("wt", [K, C, NTAP * C], bf16, kind="ExternalInput").ap()
    se1_d = nc.dram_tensor("se1", [C, HID], f32, kind="ExternalInput").ap()
    se2_d = nc.dram_tensor("se2", [HID, K], f32, kind="ExternalInput").ap()
    bias_d = nc.dram_tensor("biasT", [K, C], f32, kind="ExternalInput").ap()
    eye_d = nc.dram_tensor("eye4", [K, K], f32, kind="ExternalInput").ap()
    gn1s_d = nc.dram_tensor("gn1s", [1, HID], f32, kind="ExternalInput").ap()
    gn1b_d = nc.dram_tensor("gn1b", [1, HID], f32, kind="ExternalInput").ap()
    gn2s_d = nc.dram_tensor("gn2s", [1, K], f32, kind="ExternalInput").ap()
    gn2b_d = nc.dram_tensor("gn2b", [1, K], f32, kind="ExternalInput").ap()
    out_d = nc.dram_tensor("out", [BSL, C, HH * WW], bf16, kind="ExternalOutput").ap()

    with tile.TileContext(nc) as tc:
        with (
            tc.tile_pool(name="xin", bufs=BSL) as xpool,
            tc.tile_pool(name="wts", bufs=1) as wpool,
            tc.tile_pool(name="small", bufs=1) as spool,
            tc.tile_pool(name="agg", bufs=2) as apool,
            tc.tile_pool(name="ostage", bufs=4) as opool,
            tc.tile_pool(name="psmall", bufs=2, space="PSUM") as pspool,
            tc.tile_pool(name="pconv", bufs=6, space="PSUM") as cpool,
        ):
            # ---- load inputs ----
            # split each sample's x DMA into partition chunks so the pieces
            # spread across DMA queues (a single dma_start serializes ~90GB/s)
            x_tiles = []
            PCH = 32
            for b in range(BSL):
                xt = xpool.tile([C, LPAD], bf16, tag=f"x{b}")
                for c in range(0, C, PCH):
                    nc.sync.dma_start(xt[c:c + PCH, :], xp_d[b][c:c + PCH, :])
                x_tiles.append(xt)

            wt_all = wpool.tile([C, K * NTAP * C], bf16)
            for k in range(K):
                nc.sync.dma_start(wt_all[:, k * NTAP * C:(k + 1) * NTAP * C], wt_d[k])

            se1_sb = spool.tile([C, HID], f32, tag="se1")
            nc.sync.dma_start(se1_sb[:], se1_d[:])
            se2_sb = spool.tile([HID, K], f32, tag="se2")
            nc.sync.dma_start(se2_sb[:], se2_d[:])
            bias_sb = spool.tile([K, C], f32, tag="biasT")
            nc.sync.dma_start(bias_sb[:], bias_d[:])
            eye_sb = spool.tile([K, K], f32, tag="eye4")
            nc.sync.dma_start(eye_sb[:], eye_d[:])
            gn1s_sb = spool.tile([1, HID], f32, tag="gn1s")
            nc.sync.dma_start(gn1s_sb[:], gn1s_d[:])
            gn1b_sb = spool.tile([1, HID], f32, tag="gn1b")
            nc.sync.dma_start(gn1b_sb[:], gn1b_d[:])
            gn2s_sb = spool.tile([1, K], f32, tag="gn2s")
            nc.sync.dma_start(gn2s_sb[:], gn2s_d[:])
            gn2b_sb = spool.tile([1, K], f32, tag="gn2b")
            nc.sync.dma_start(gn2b_sb[:], gn2b_d[:])

            # ---- SE attention ----
            # pooled_T[ci, b] = sum_hw x[b, ci] (zeros in padding don't affect sum;
            # the 1/4096 mean factor is pre-folded into se1 host-side).
            # Each sample's sum is split across DVE (tensor_reduce) and ACT
            # (copy with accumulator) so the four samples don't serialize on DVE.
            HALF = LPAD // 2
            p_dve = spool.tile([C, BSL], f32, tag="p_dve")
            p_act = spool.tile([C, BSL], f32, tag="p_act")
            for b in range(BSL):
                nc.vector.tensor_reduce(
                    out=p_dve[:, b:b + 1], in_=x_tiles[b][:, 0:HALF],
                    axis=mybir.AxisListType.X, op=Alu.add)
                red_sc = spool.tile([C, LPAD - HALF], bf16, tag=f"redsc{b % 2}")
                nc.scalar.activation(red_sc[:], x_tiles[b][:, HALF:LPAD],
                                     Act.Identity, accum_out=p_act[:, b:b + 1])
            pooled = spool.tile([C, BSL], f32, tag="pooled")
            nc.vector.tensor_add(pooled[:], p_dve[:], p_act[:])

            def groupnorm_rows(h_ps, n, scale_sb, bias_sb_, tag):
                """GroupNorm(1) over the free dim of a [BSL, n] PSUM tile.
                Returns normalized [BSL, n] f32 SBUF tile (pre-activation)."""
                msum = spool.tile([BSL, 1], f32, tag=f"{tag}_msum")
                nc.vector.tensor_reduce(out=msum[:], in_=h_ps[:],
                                        axis=mybir.AxisListType.X, op=Alu.add)
                mean = spool.tile([BSL, 1], f32, tag=f"{tag}_mean")
                nc.vector.tensor_scalar_mul(mean[:], msum[:], 1.0 / n)
                cent = spool.tile([BSL, n], f32, tag=f"{tag}_cent")
                nc.vector.tensor_scalar_sub(cent[:], h_ps[:], mean[:, 0:1])
                sq = spool.tile([BSL, n], f32, tag=f"{tag}_sq")
                vsum = spool.tile([BSL, 1], f32, tag=f"{tag}_vsum")
                nc.scalar.activation(sq[:], cent[:], Act.Square,
                                     accum_out=vsum[:])
                # rstd = sqrt(1/(var+eps)) all on DVE: reciprocal + globally
                # convergent sqrt-Newton — keeps the whole SE on one ACT table
                # (DVE int/bit ops are not HW-legal, and ACT Sqrt would force
                # table reloads around exp/tanh)
                s = spool.tile([BSL, 1], f32, tag=f"{tag}_s")
                nc.vector.tensor_scalar(s[:], vsum[:], 1.0 / n, EPS,
                                        op0=Alu.mult, op1=Alu.add)
                r = spool.tile([BSL, 1], f32, tag=f"{tag}_r")
                nc.vector.reciprocal(r[:], s[:])
                rh = spool.tile([BSL, 1], f32, tag=f"{tag}_rh")
                nc.vector.tensor_scalar_mul(rh[:], r[:], 0.5)
                z = spool.tile([BSL, 1], f32, tag=f"{tag}_z")
                nc.vector.tensor_scalar(z[:], r[:], 1.0 / 64, 1.0,
                                        op0=Alu.mult, op1=Alu.max)
                for it in range(7):
                    u = spool.tile([BSL, 1], f32, tag=f"{tag}_u{it}")
                    nc.vector.reciprocal(u[:], z[:])
                    tm = spool.tile([BSL, 1], f32, tag=f"{tag}_tm{it}")
                    nc.vector.tensor_mul(tm[:], rh[:], u[:])
                    zn = spool.tile([BSL, 1], f32, tag=f"{tag}_zn{it}")
                    nc.vector.scalar_tensor_tensor(
                        out=zn[:], in0=z[:], scalar=0.5, in1=tm[:],
                        op0=Alu.mult, op1=Alu.add)
                    z = zn
                rstd = z
                norm = spool.tile([BSL, n], f32, tag=f"{tag}_norm")
                nc.vector.tensor_scalar_mul(norm[:], cent[:], rstd[:, 0:1])
                # elementwise affine along free dim (broadcast params to BSL partitions)
                s_bc = spool.tile([BSL, n], f32, tag=f"{tag}_sbc")
                nc.gpsimd.partition_broadcast(s_bc[:], scale_sb[:])
                b_bc = spool.tile([BSL, n], f32, tag=f"{tag}_bbc")
                nc.gpsimd.partition_broadcast(b_bc[:], bias_sb_[:])
                affn = spool.tile([BSL, n], f32, tag=f"{tag}_affn")
                nc.vector.tensor_mul(affn[:], norm[:], s_bc[:])
                affnb = spool.tile([BSL, n], f32, tag=f"{tag}_affnb")
                nc.vector.tensor_add(affnb[:], affn[:], b_bc[:])
                return affnb

            # h1 = pooled @ (se_w1/4096).T  -> [BSL, HID] (f32 matmul, tiny)
            h1_ps = pspool.tile([BSL, HID], f32, tag="seps")
            nc.tensor.matmul(h1_ps[:], lhsT=pooled[:], rhs=se1_sb[:])
            h1n = groupnorm_rows(h1_ps, HID, gn1s_sb, gn1b_sb, "gn1")

            # mish(v) = v * tanh(softplus(v)) = v * (1 - 2/((1+e^v)^2 + 1))
            # (one ACT exp; the rest stays on DVE — no extra table switches)
            ev = spool.tile([BSL, HID], f32, tag="mish_ev")
            nc.scalar.activation(ev[:], h1n[:], Act.Exp)
            ep1 = spool.tile([BSL, HID], f32, tag="mish_ep1")
            nc.vector.tensor_scalar_add(ep1[:], ev[:], 1.0)
            q = spool.tile([BSL, HID], f32, tag="mish_q")
            nc.vector.tensor_mul(q[:], ep1[:], ep1[:])
            qp1 = spool.tile([BSL, HID], f32, tag="mish_qp1")
            nc.vector.tensor_scalar_add(qp1[:], q[:], 1.0)
            rq = spool.tile([BSL, HID], f32, tag="mish_rq")
            nc.vector.reciprocal(rq[:], qp1[:])
            th = spool.tile([BSL, HID], f32, tag="mish_th")
            nc.vector.tensor_scalar(th[:], rq[:], -2.0, 1.0,
                                    op0=Alu.mult, op1=Alu.add)
            h1m = spool.tile([BSL, HID], f32, tag="h1m")
            nc.vector.tensor_mul(h1m[:], h1n[:], th[:])

            # h2 = h1m @ se_w2.T -> [BSL, K]; needs h1m.T as lhsT
            h1mT_ps = pspool.tile([HID, BSL], f32, tag="seps")
            nc.tensor.transpose(h1mT_ps[:], h1m[:], eye_sb[:])
            h1mT = spool.tile([HID, BSL], f32, tag="h1mT")
            nc.scalar.copy(h1mT[:], h1mT_ps[:])
            h2_ps = pspool.tile([BSL, K], f32, tag="seps")
            nc.tensor.matmul(h2_ps[:], lhsT=h1mT[:], rhs=se2_sb[:])
            h2n = groupnorm_rows(h2_ps, K, gn2s_sb, gn2b_sb, "gn2")
            # sigmoid(z) = 0.5 * (1 + tanh(z/2)); tanh shares the exp ACT table
            tnh = spool.tile([BSL, K], f32, tag="att_tnh")
            nc.scalar.activation(tnh[:], h2n[:], Act.Tanh, scale=0.5)
            att = spool.tile([BSL, K], f32, tag="att")
            nc.vector.tensor_scalar(att[:], tnh[:], 0.5, 0.5,
                                    op0=Alu.mult, op1=Alu.add)

            # agg_b.T[co, b] = bias.T @ att.T  (lhsT = bias [k, co], rhs = att.T [k, b])
            attT_ps = pspool.tile([K, BSL], f32, tag="seps")
            nc.tensor.transpose(attT_ps[:], att[:], eye_sb[:])
            attT = spool.tile([K, BSL], f32, tag="attT")
            nc.scalar.copy(attT[:], attT_ps[:])
            aggb_ps = pspool.tile([C, BSL], f32, tag="seps")
            nc.tensor.matmul(aggb_ps[:], lhsT=bias_sb[:], rhs=attT[:])
            aggb = spool.tile([C, BSL], f32, tag="aggb")
            nc.scalar.copy(aggb[:], aggb_ps[:])

            # broadcast att to all 128 partitions: att_bc[p, b*K+k] = att[b, k]
            att_flat = spool.tile([1, BSL * K], f32, tag="attflat")
            nc.sync.dma_start(att_flat[:], att[:])
            att_bc = spool.tile([C, BSL * K], f32, tag="attbc")
            nc.gpsimd.partition_broadcast(att_bc[:], att_flat[:])

            # ---- per-sample aggregation + conv ----
            W1 = NTAP * C  # 1152 free elems per base kernel
            NCHUNK = 3     # aggregate in 3-tap column chunks so conv block 0
            CW = W1 // NCHUNK  # can start after the first chunk, not all 1152
            for b in range(BSL):
                # agg[ci, tap*C+co] = sum_k att[b,k] * wT[k][ci, tap*C+co]
                agg_chunks = [None] * NCHUNK
                for c in (1, 0, 2):  # conv consumes dh=0 taps (chunk 1) first
                    prev = None
                    for k in range(K):
                        cur = apool.tile([C, CW], bf16, tag=f"agg{c}_{k}")
                        sc = att_bc[:, b * K + k:b * K + k + 1]
                        wk = wt_all[:, k * W1 + c * CW:k * W1 + (c + 1) * CW]
                        if prev is None:
                            nc.vector.tensor_scalar_mul(cur[:], wk, sc)
                        else:
                            nc.vector.scalar_tensor_tensor(
                                out=cur[:], in0=wk, scalar=sc, in1=prev[:],
                                op0=Alu.mult, op1=Alu.add)
                        prev = cur
                    agg_chunks[c] = prev

                def agg_tap(tap):
                    c, r = divmod(tap * C, CW)
                    return agg_chunks[c][:, r:r + C]

                for blk in range(NBLK):
                    h0 = blk * RB
                    ps = cpool.tile([C, RB * WW], f32, tag="convps")
                    ti = 0
                    taps = [(dh, dw) for dh in (0, -1, 1) for dw in (-1, 0, 1)]
                    for (dh, dw) in taps:
                        tt = 1 if h0 + dh < 0 else 0
                        bt = 1 if h0 + RB - 1 + dh > HH - 1 else 0
                        nr = RB - tt - bt
                        tap = (dh + 1) * 3 + (dw + 1)
                        start = 1 + (h0 + tt + dh) * LROW + dw
                        rhs = (x_tiles[b][:, start:start + nr * LROW]
                               .rearrange("p (r c) -> p r c", c=LROW)[:, :, 0:WW])
                        nc.tensor.matmul(
                            ps[:, tt * WW:(tt + nr) * WW],
                            lhsT=agg_tap(tap),
                            rhs=rhs,
                            start=(ti == 0), stop=(ti == NTAP - 1))
                        ti += 1
                    osb = opool.tile([C, RB * WW], bf16)
                    nc.scalar.activation(osb[:], ps[:], Act.Identity,
                                         bias=aggb[:, b:b + 1], scale=1.0)
                    nc.sync.dma_start(out_d[b][:, h0 * WW:(h0 + RB) * WW], osb[:])

    nc.compile()
    return nc


def _stage_inputs(x, weight, bias, se_w1, gn1_scale, gn1_bias, se_w2,
                  gn2_scale, gn2_bias):
    """Host-side layout staging: shard, pad, transpose, cast. Returns in_maps."""
    bf16 = ml_dtypes.bfloat16

    # zero-padded x: per (b, ci) buffer of length LPAD; element (h, w) lives at
    # 1 + h*LROW + w, so w-1/w+64 shifts read zeros and row shifts stay in bounds.
    xp = np.zeros((BS, C, LPAD), dtype=bf16)
    xp_view = xp[:, :, 1:1 + HH * LROW].reshape(BS, C, HH, LROW)
    xp_view[:, :, :, :WW] = x.astype(bf16)

    # weight [k, o, i, h, w] -> lhsT layout [k, i, (h*3+w)*C + o]
    wt = np.ascontiguousarray(weight.transpose(0, 2, 3, 4, 1)).reshape(K, C, NTAP * C).astype(bf16)

    se1 = np.ascontiguousarray((se_w1 / float(HH * WW)).T).astype(np.float32)   # [C, HID]
    se2 = np.ascontiguousarray(se_w2.T).astype(np.float32)                      # [HID, K]
    biasT = np.ascontiguousarray(bias).astype(np.float32)                       # [K, C]
    eye4 = np.eye(K, dtype=np.float32)
    g1s = gn1_scale.reshape(1, HID).astype(np.float32)
    g1b = gn1_bias.reshape(1, HID).astype(np.float32)
    g2s = gn2_scale.reshape(1, K).astype(np.float32)
    g2b = gn2_bias.reshape(1, K).astype(np.float32)

    in_maps = []
    for i in range(N_CORES):
        in_maps.append({
            "xp": np.ascontiguousarray(xp[i * BSL:(i + 1) * BSL]),
            "wt": wt, "se1": se1, "se2": se2, "biasT": biasT, "eye4": eye4,
            "gn1s": g1s, "gn1b": g1b, "gn2s": g2s, "gn2b": g2b,
        })
    return in_maps


def kernel(x, weight, bias, se_w1, gn1_scale, gn1_bias, se_w2, gn2_scale,
           gn2_bias):
    global LAST_EXEC_NS
    x = np.asarray(x, dtype=np.float32)
    weight = np.asarray(weight, dtype=np.float32)
    bias = np.asarray(bias, dtype=np.float32)
    se_w1 = np.asarray(se_w1, dtype=np.float32)
    gn1_scale = np.asarray(gn1_scale, dtype=np.float32)
    gn1_bias = np.asarray(gn1_bias, dtype=np.float32)
    se_w2 = np.asarray(se_w2, dtype=np.float32)
    gn2_scale = np.asarray(gn2_scale, dtype=np.float32)
    gn2_bias = np.asarray(gn2_bias, dtype=np.float32)

    if "nc" not in _cache:
        _cache["nc"] = _build_nc()
    nc = _cache["nc"]

    in_maps = _stage_inputs(x, weight, bias, se_w1, gn1_scale, gn1_bias,
                            se_w2, gn2_scale, gn2_bias)

    trace = bool(int(os.environ.get("BASS_KERNEL_TRACE", "0")))
    if trace:
        trace = _install_trace_hook()

    from concourse.bass_utils import run_bass_kernel_spmd
    res = run_bass_kernel_spmd(nc, in_maps, core_ids=list(range(N_CORES)),
                               trace=trace)
    LAST_EXEC_NS = res.exec_time_ns

    out = np.empty((BS, C, HH, WW), dtype=np.float32)
    for i in range(N_CORES):
        out[i * BSL:(i + 1) * BSL] = (
            res.results[i]["out"].astype(np.float32).reshape(BSL, C, HH, WW))
    return out
